# revision 77
# baseline (speedup 1.0000x reference)
"""Trainium2 Bass kernel for nn_GraphVertExtraLinModel.

Model (per sample n, GS=4 graph channels, M=64 nodes):
  layer: h <- max_g relu(G[n,g] @ (h @ W[g].T + b[g]))  (+ residual for l>=1)
  head:  out = relu(h @ lin1_w.T + lin1_b) @ lin2_w.T + lin2_b

Sharding: data-parallel over N=128 -> 16 samples per core, weights replicated.
No collectives needed (the max-aggregation is over GS inside each sample).

Per-core layout (tokens = 16*64 = 1024, tiled 8 x 128; h kept FEATURE-major):
  mp    [tok, (g-pair, p)] = hT.T @ W   (layers>=1: split-fp8 DoubleRow:
                                         h@W = h8@W8 + h8@dW8 + dh8@W8, all
                                         e4m3 with power-of-2 scales, 6 DR
                                         matmuls of K=256 at 0.5 cyc/row =
                                         3/4 the f32r cost; residual-quantized
                                         dW/dh keep bf16-class accuracy.
                                         L0 stays f32r. Two g channels packed
                                         per 2-bank PSUM tile.)
  ms    [tok, p] = mp/(sh*sw) + b       (descale+bias ride the PSUM->SBUF
                                         copy, bf16; pair 0|1 on DVE stt,
                                         pair 2|3 via Act-copy(scale) +
                                         GpSimd-add chain)
  xoT   [p, tok] = ms.T @ Gblk          (G-matmul flipped: ms stationary, G
                                         moving in bf16 so the 128-free matmul
                                         runs 1 cyc/row; output lands feature-
                                         major -> NO transposes anywhere)
  xr    = relu(xoT) bf16 on Act         (relu commutes with max)
  h'    = max_g xr + h                  (bf16 max tree on DVE 2x mode,
                                         residual add on GpSimd)
  h8    = fp8(qs*h); dh8 = fp8(qs*h-h8) (quant chain on DVE, feeds next layer)
fp8 h-scales are calibrated at runtime from a 2-sample host forward pass
(power-of-2 scales, 8x margin) before the program is built.
G is pre-transposed + block-diag packed (2 samples per 128x128 tile) on host.
PE program order interleaves mm1[t] with the G-matmuls of t-1 so the helper
engines' ms copies never stall the PE.
"""

import math

import numpy as np
import ml_dtypes
from contextlib import ExitStack

import concourse.bass as bass
import concourse.tile as tile
from concourse import bacc, mybir
from concourse.bass_utils import run_bass_kernel_spmd
from concourse.alu_op_type import AluOpType

F32 = mybir.dt.float32
F32R = mybir.dt.float32r
BF16 = mybir.dt.bfloat16
F8 = mybir.dt.float8e4
RELU = mybir.ActivationFunctionType.Relu
COPY = mybir.ActivationFunctionType.Copy
DR = mybir.MatmulPerfMode.DoubleRow
E4M3 = ml_dtypes.float8_e4m3fn

N_CORES = 8
N_FULL = 128
N_LOC = N_FULL // N_CORES   # 16 samples per core
GS = 4
M = 64
C_IN = 128
D = 512
L = 8
TOK = N_LOC * M             # 1024 tokens per core
NT = TOK // 128             # 8 token tiles
KD = D // 128               # 4 contraction tiles for D
D2 = 2 * D                  # paired g-channel width


def _build_program(qs, sw):
    """qs[l]: fp8 scale of the h produced by layer l (l=0..L-2);
    sw[l]: fp8 scale of layer l's weights (l=1..L-1, index l-1)."""
    nc = bacc.Bacc(
        "TRN2",
        target_bir_lowering=False,
        debug=False,
        enable_asserts=False,
        num_devices=N_CORES,
    )

    xT_d = nc.dram_tensor("xT", [C_IN, TOK], F32R, kind="ExternalInput").ap()
    g_d = nc.dram_tensor("gsb", [128, GS * NT * 128], BF16, kind="ExternalInput").ap()
    w0_d = nc.dram_tensor("w0", [128, GS * D], F32R, kind="ExternalInput").ap()
    b0_d = nc.dram_tensor("b0", [128, GS * D], BF16, kind="ExternalInput").ap()
    w8_d = nc.dram_tensor("w8", [L - 1, 128, GS * KD * D], F8, kind="ExternalInput").ap()
    dw8_d = nc.dram_tensor(
        "dw8", [L - 1, 128, GS * KD * D], F8, kind="ExternalInput"
    ).ap()
    b_d = nc.dram_tensor("b", [L - 1, 128, GS * D], BF16, kind="ExternalInput").ap()
    # pre-scaled bias rows (rank-1 G-matmul bias for g2/g3) + G column sums
    br_d = nc.dram_tensor("brow", [1, L * GS * D], BF16, kind="ExternalInput").ap()
    grs_d = nc.dram_tensor("grs", [1, NT * GS * 128], BF16, kind="ExternalInput").ap()
    l1w_d = nc.dram_tensor("lin1", [128, KD * 128], F32R, kind="ExternalInput").ap()
    l1b_d = nc.dram_tensor("lin1b", [128, 1], F32, kind="ExternalInput").ap()
    l2w_d = nc.dram_tensor("lin2", [128, 1], F32R, kind="ExternalInput").ap()
    out_d = nc.dram_tensor("out", [1, TOK], F32, kind="ExternalOutput").ap()

    with tile.TileContext(nc) as tc, ExitStack() as ctx:
        const = ctx.enter_context(tc.tile_pool(name="const", bufs=1))
        wpool = ctx.enter_context(tc.tile_pool(name="w", bufs=2))
        dwpool = ctx.enter_context(tc.tile_pool(name="dw", bufs=2))
        bpool = ctx.enter_context(tc.tile_pool(name="b", bufs=2))
        hpool = ctx.enter_context(tc.tile_pool(name="h", bufs=2))
        h8pool = ctx.enter_context(tc.tile_pool(name="h8", bufs=2))
        dh8pool = ctx.enter_context(tc.tile_pool(name="dh8", bufs=2))
        mspool = ctx.enter_context(tc.tile_pool(name="ms", bufs=10))
        mtmp = ctx.enter_context(tc.tile_pool(name="mt", bufs=10))
        # per-g mp tiles (1 PSUM bank each, 4-deep ring) give each slot 4x the
        # turnaround budget vs a 2-deep ring of paired tiles
        mpsum = ctx.enter_context(tc.tile_pool(name="mpsum", bufs=4, space="PSUM"))
        xpsum = ctx.enter_context(tc.tile_pool(name="xpsum", bufs=2, space="PSUM"))

        # startup-critical DMAs first: the first mm1 needs xT and w0[g0|g1];
        # gsb (t-major) is needed by the first G-matmul flush
        xsb = const.tile([128, TOK], F32R, tag="xsb")
        nc.sync.dma_start(out=xsb[:, 0:256], in_=xT_d[:, 0:256])
        wsb0 = wpool.tile([128, GS * D], F32R, tag="w0")
        bsb0 = bpool.tile([128, GS * D], BF16, tag="b")
        gsb = const.tile([128, GS * NT * 128], BF16, tag="gsb")
        # startup order: mm1(0,0) deps first (xa, w0; b0 early for the DVE ms
        # chain), then the first-flush deps (gsb[t0], grs, brow[L0]) ahead of
        # xb and the big gsb remainder -- PE's in-order stream blocks on
        # flush(0,0) before it reaches mm1(0,2) anyway
        nc.sync.dma_start(out=wsb0[:, 0:D], in_=w0_d[:, 0:D])
        nc.sync.dma_start(out=wsb0[:, D:D2], in_=w0_d[:, D:D2])
        nc.sync.dma_start(out=bsb0[:, 0:D2], in_=b0_d[:, 0:D2])
        nc.sync.dma_start(out=wsb0[:, D2 : 3 * D], in_=w0_d[:, D2 : 3 * D])
        nc.sync.dma_start(out=wsb0[:, 3 * D :], in_=w0_d[:, 3 * D :])
        nc.sync.dma_start(out=bsb0[:, D2:], in_=b0_d[:, D2:])
        brsb = const.tile([1, L * GS * D], BF16, tag="brow")
        grsb = const.tile([1, NT * GS * 128], BF16, tag="grs")
        nc.sync.dma_start(out=gsb[:, 0:1024], in_=g_d[:, 0:1024])
        nc.sync.dma_start(out=grsb[:], in_=grs_d)
        nc.sync.dma_start(out=brsb[0:1, 0 : GS * D], in_=br_d[0:1, 0 : GS * D])
        nc.sync.dma_start(out=xsb[:, 256:TOK], in_=xT_d[:, 256:TOK])
        nc.sync.dma_start(out=gsb[:, 1024:], in_=g_d[:, 1024:])
        nc.sync.dma_start(out=brsb[0:1, GS * D :], in_=br_d[0:1, GS * D :])
        l1sb = const.tile([128, KD * 128], F32R, tag="l1w")
        l1b = const.tile([128, 1], F32, tag="l1b")
        l2sb = const.tile([128, 1], F32R, tag="l2w")
        osb = const.tile([1, TOK], F32, tag="osb")

        # pending G-matmul + max-tree work: emitted one t-iteration late so the
        # PE never waits on the helper engines' ms copies
        pending = []

        def flush():
            if not pending:
                return
            layer, t, ms_list, h_new, h_prev, h8_new, dh8_new, inv = pending.pop(0)
            xrs = []
            for half in range(2):          # halves: (g0|g1), (g2|g3)
                xo = xpsum.tile([128, D2], F32, tag="xo")
                for gi in range(2):
                    g = half * 2 + gi
                    for p in range(KD):
                        xslice = xo[:, gi * D + p * 128 : gi * D + (p + 1) * 128]
                        if g >= 2:
                            # bias as rank-1: b_scaled[p] x colsum(G)[i]; the
                            # ms copies for g2/g3 are then pure Act copies
                            nc.tensor.matmul(
                                xslice,
                                brsb[
                                    0:1,
                                    layer * GS * D + g * D + p * 128 : layer * GS * D
                                    + g * D
                                    + (p + 1) * 128,
                                ],
                                grsb[
                                    0:1, (t * GS + g) * 128 : (t * GS + g + 1) * 128
                                ],
                                start=True,
                                stop=False,
                            )
                        nc.tensor.matmul(
                            xslice,
                            ms_list[g][:, p * 128 : (p + 1) * 128],
                            gsb[:, (t * GS + g) * 128 : (t * GS + g + 1) * 128],
                            start=(g < 2),
                            stop=True,
                        )
                xr = mtmp.tile([128, D2], BF16, tag="mt")
                # the fp8 descale rides the relu's scale (relu(x*inv), inv>0)
                nc.scalar.activation(xr[:], xo[:], func=RELU, scale=inv)
                xrs.append(xr)
            m01 = mtmp.tile([128, D], BF16, tag="mt")
            nc.vector.tensor_tensor(
                m01[:], xrs[0][:, 0:D], xrs[0][:, D:D2], op=AluOpType.max
            )
            m23 = mtmp.tile([128, D], BF16, tag="mt")
            nc.vector.tensor_tensor(
                m23[:], xrs[1][:, 0:D], xrs[1][:, D:D2], op=AluOpType.max
            )
            hs = h_new[:, t * D : (t + 1) * D]
            if h_prev is None:
                nc.vector.tensor_tensor(hs, m01[:], m23[:], op=AluOpType.max)
            else:
                u = mtmp.tile([128, D], BF16, tag="mt")
                nc.vector.tensor_tensor(u[:], m01[:], m23[:], op=AluOpType.max)
                # last flush feeds the final head block: chain the residual on
                # DVE (no GpSimd launch latency)
                eng = nc.vector if (layer == L - 1 and t == NT - 1) else nc.gpsimd
                eng.tensor_tensor(
                    hs, u[:], h_prev[:, t * D : (t + 1) * D], op=AluOpType.add
                )
            if h8_new is not None:
                # fp8 quant chain feeding the next layer's DoubleRow matmuls:
                # h8 = fp8(qs*h); dh8 = fp8(qs*h - h8). In L0 the quant rides
                # GpSimd (idle there: no residual); later layers use DVE.
                h8s = h8_new[:, t * D : (t + 1) * D]
                nc.vector.tensor_scalar_mul(h8s, hs, qs[layer])
                nc.vector.scalar_tensor_tensor(
                    dh8_new[:, t * D : (t + 1) * D],
                    hs,
                    qs[layer],
                    h8s,
                    op0=AluOpType.mult,
                    op1=AluOpType.subtract,
                )

        def emit_head_block(h_tile, tb):
            # 256-token head block: free dim stays >= 256 for full f32r rate
            h3 = h_tile[:].rearrange("p (t k) -> p t k", t=NT)
            p1 = mpsum.tile([128, 256], F32, tag="mp")
            for c in range(KD):
                nc.tensor.matmul(
                    p1[:],
                    l1sb[:, c * 128 : (c + 1) * 128],
                    h3[:, tb * 2 : (tb + 1) * 2, c * 128 : (c + 1) * 128],
                    start=(c == 0),
                    stop=(c == KD - 1),
                )
            x1 = mtmp.tile([128, 256], F32R, tag="mt")
            nc.scalar.activation(x1[:], p1[:], func=RELU, bias=l1b[:])
            p2 = xpsum.tile([1, 256], F32, tag="xo")
            nc.tensor.matmul(p2[:], l2sb[:], x1[:], start=True, stop=True)
            nc.vector.tensor_copy(osb[0:1, tb * 256 : (tb + 1) * 256], p2[:])

        h_prev = None
        h8_prev = dh8_prev = None
        for layer in range(L):
            if layer == 0:
                wsb, bsb = wsb0, bsb0
                dwsb = None
                inv = 1.0
            else:
                # per-g chunks so the first mm1 of the layer isn't gated on
                # the full weight transfer
                wsb = wpool.tile([128, GS * KD * D], F8, tag="w8")
                dwsb = dwpool.tile([128, GS * KD * D], F8, tag="dw8")
                for g in range(GS):
                    sl = slice(g * KD * D, (g + 1) * KD * D)
                    nc.sync.dma_start(out=wsb[:, sl], in_=w8_d[layer - 1][:, sl])
                    nc.sync.dma_start(out=dwsb[:, sl], in_=dw8_d[layer - 1][:, sl])
                bsb = bpool.tile([128, GS * D], BF16, tag="b")
                nc.sync.dma_start(out=bsb[:], in_=b_d[layer - 1])
                inv = 1.0 / (qs[layer - 1] * sw[layer - 1])
            if layer == 1:
                # head weights: needed only at the very end
                nc.sync.dma_start(out=l1sb[:], in_=l1w_d)
                nc.sync.dma_start(out=l1b[:], in_=l1b_d)
                nc.sync.dma_start(out=l2sb[:], in_=l2w_d)

            h_new = hpool.tile([128, NT * D], F32R, tag="h")
            if layer < L - 1:
                h8_new = h8pool.tile([128, NT * D], F8, tag="h8")
                dh8_new = dh8pool.tile([128, NT * D], F8, tag="dh8")
            else:
                h8_new = dh8_new = None
            for t in range(NT):
                ms_list = []
                for g in range(GS):
                    mp = mpsum.tile([128, D], F32, tag="mp")
                    if layer == 0:
                        nc.tensor.matmul(
                            mp[:],
                            xsb[:, t * 128 : (t + 1) * 128],
                            wsb[:, g * D : (g + 1) * D],
                            start=True,
                            stop=True,
                        )
                    else:
                        # split-fp8: h8@W8 + h8@dW8 + dh8@W8, two K=256
                        # DoubleRow passes each (q selects c-tile pair)
                        terms = (
                            (h8_prev, wsb),
                            (h8_prev, dwsb),
                            (dh8_prev, wsb),
                        )
                        for ti, (hsrc, wsrc) in enumerate(terms):
                            for q in range(2):
                                lhs = hsrc[
                                    :,
                                    t * D + 2 * q * 128 : t * D + (2 * q + 2) * 128,
                                ].rearrange("p (i m) -> p i m", i=2)
                                rhs = wsrc[
                                    :, (g * 2 + q) * 1024 : (g * 2 + q + 1) * 1024
                                ].rearrange("p (i o) -> p i o", i=2)
                                nc.tensor.matmul(
                                    mp[:],
                                    lhs,
                                    rhs,
                                    start=(ti == 0 and q == 0),
                                    stop=(ti == 2 and q == 1),
                                    perf_mode=DR,
                                )
                    ms = mspool.tile([128, D], BF16, tag="ms")
                    if g < 2:
                        # bias (host pre-scaled by qs*sw) rides the copy
                        nc.vector.tensor_tensor(
                            ms[:], mp[:], bsb[:, g * D : (g + 1) * D], op=AluOpType.add
                        )
                    elif layer == L - 1 and t >= NT - 2 and g == 3:
                        # tail: let the last copy run on DVE in parallel with
                        # Act's g2 copy so ms completes sooner
                        nc.vector.tensor_copy(ms[:], mp[:])
                    else:
                        # pure copy; bias comes via the rank-1 G-matmul
                        nc.scalar.activation(ms[:], mp[:], func=COPY)
                    ms_list.append(ms)
                flush()
                pending.append(
                    (layer, t, ms_list, h_new, h_prev, h8_new, dh8_new, inv)
                )
            h_prev = h_new
            h8_prev, dh8_prev = h8_new, dh8_new
        flush()
        for tb in range(4):
            emit_head_block(h_prev, tb)
        nc.sync.dma_start(out=out_d[:], in_=osb[:])

    nc.compile()
    return nc


_NC = None


def _get_nc(qs=None, sw=None):
    global _NC
    if _NC is None:
        assert qs is not None and sw is not None
        _NC = _build_program(qs, sw)
    return _NC


def _pow2_scale(maxabs, margin):
    return 2.0 ** math.floor(math.log2(448.0 / (maxabs * margin)))


def _calibrate(G, x, W0, b0, W, b, n_samples=2):
    """Host forward pass on a couple of samples -> per-layer h max-abs."""
    Gs = G[:n_samples]
    h = x[:n_samples]
    qs = []
    for layer in range(L - 1):
        if layer == 0:
            Wl, bl = W0, b0
        else:
            Wl, bl = W[layer - 1], b[layer - 1]
        multi = np.einsum("nmc,gpc->gnmp", h, Wl, optimize=True) + bl[:, None, None, :]
        xo = np.einsum("ngij,gnjp->ngip", Gs, multi, optimize=True)
        hnew = np.maximum(xo, 0.0).max(axis=1)
        h = hnew + h if layer > 0 else hnew
        qs.append(_pow2_scale(np.abs(h).max(), 8.0))
    return qs


def _prep_in_maps(G, x, W0, b0, W, b, lin1_w, lin1_b, lin2_w, lin2_b, sw, qs):
    BF = ml_dtypes.bfloat16
    G = np.ascontiguousarray(np.asarray(G, dtype=np.float32))
    x = np.ascontiguousarray(np.asarray(x, dtype=np.float32))
    W0 = np.asarray(W0, dtype=np.float32)
    b0 = np.asarray(b0, dtype=np.float32)
    W = np.asarray(W, dtype=np.float32)
    b = np.asarray(b, dtype=np.float32)
    lin1_w = np.asarray(lin1_w, dtype=np.float32)
    lin1_b = np.asarray(lin1_b, dtype=np.float32)
    lin2_w = np.asarray(lin2_w, dtype=np.float32)

    # w0: [c_local, (g, p)] with row = input channel c (f32r, layer 0)
    w0f = np.ascontiguousarray(W0.transpose(2, 0, 1).reshape(C_IN, GS * D))
    b0f = np.ascontiguousarray(
        np.broadcast_to(b0.reshape(1, GS * D), (128, GS * D))
    ).astype(BF)
    # layers >= 1: split-fp8 in DoubleRow layout [l, pc, (g, q, i, pout)]
    # with contraction c = (2q + i)*128 + pc
    Wd = W.reshape(L - 1, GS, D, 2, 2, 128).transpose(0, 5, 1, 3, 4, 2)
    Wd = np.ascontiguousarray(Wd.reshape(L - 1, 128, GS * KD * D))
    w8 = np.empty_like(Wd, dtype=E4M3)
    dw8 = np.empty_like(Wd, dtype=E4M3)
    for l in range(L - 1):
        ws = Wd[l] * sw[l]
        w8[l] = ws.astype(E4M3)
        dw8[l] = (ws - w8[l].astype(np.float32)).astype(E4M3)
    # biases pre-scaled by the layer's fp8 scale product (descale rides relu)
    bscale = np.array([qs[l] * sw[l] for l in range(L - 1)], np.float32)
    bsc = b.reshape(L - 1, GS * D) * bscale[:, None]
    bf = np.ascontiguousarray(
        np.broadcast_to(bsc.reshape(L - 1, 1, GS * D), (L - 1, 128, GS * D))
    ).astype(BF)
    brow = np.concatenate(
        [b0.reshape(1, GS * D), bsc], axis=0
    ).reshape(1, L * GS * D).astype(BF)
    # lin1: [c_local, (ctile, e)]
    l1f = np.ascontiguousarray(
        lin1_w.T.reshape(KD, 128, 128).transpose(1, 0, 2).reshape(128, KD * 128)
    )
    l1b = np.ascontiguousarray(lin1_b.reshape(128, 1))
    l2f = np.ascontiguousarray(lin2_w.T)  # [128, 1]

    in_maps = []
    for cix in range(N_CORES):
        Gc = G[cix * N_LOC : (cix + 1) * N_LOC]                      # [16,4,64,64]
        xs = x[cix * N_LOC : (cix + 1) * N_LOC]                      # [16,64,128]
        xT = np.ascontiguousarray(xs.reshape(TOK, C_IN).T)           # [128,1024]
        Gt = Gc.transpose(1, 0, 3, 2)                                # [4,16,64j,64i]
        gblk = np.zeros((GS, NT, 128, 128), np.float32)
        gblk[:, :, 0:64, 0:64] = Gt[:, 0::2]
        gblk[:, :, 64:128, 64:128] = Gt[:, 1::2]
        gf = np.ascontiguousarray(
            gblk.transpose(2, 1, 0, 3).reshape(128, NT * GS * 128)
        ).astype(BF)
        grs = np.ascontiguousarray(
            gblk.sum(axis=2).transpose(1, 0, 2).reshape(1, NT * GS * 128)
        ).astype(BF)
        in_maps.append(
            {
                "xT": xT,
                "gsb": gf,
                "w0": w0f,
                "b0": b0f,
                "w8": w8,
                "dw8": dw8,
                "b": bf,
                "brow": brow,
                "grs": grs,
                "lin1": l1f,
                "lin1b": l1b,
                "lin2": l2f,
            }
        )

    return in_maps


def kernel(G, x, W0, b0, W, b, lin1_w, lin1_b, lin2_w, lin2_b, _trace=False):
    G = np.asarray(G, dtype=np.float32)
    x = np.asarray(x, dtype=np.float32)
    W = np.asarray(W, dtype=np.float32)
    W0 = np.asarray(W0, dtype=np.float32)
    b0 = np.asarray(b0, dtype=np.float32)
    b = np.asarray(b, dtype=np.float32)
    lin2_b = np.asarray(lin2_b, dtype=np.float32)
    sw = [_pow2_scale(np.abs(W[l]).max(), 2.0) for l in range(L - 1)]
    qs = _calibrate(G, x, W0, b0, W, b)
    in_maps = _prep_in_maps(
        G, x, W0, b0, W, b, lin1_w, lin1_b, lin2_w, lin2_b, sw, qs
    )
    res = run_bass_kernel_spmd(
        _get_nc(qs, sw), in_maps, list(range(N_CORES)), trace=_trace
    )
    kernel._last_results = res
    out = np.concatenate(
        [res.results[c]["out"].reshape(N_LOC, M, 1) for c in range(N_CORES)], axis=0
    )
    return (out + lin2_b[0]).astype(np.float32)
